# revision 14
# baseline (speedup 1.0000x reference)
"""MoE top-1 routing with expert capacity (nn_ExpertAllocation) on 8 TRN2 cores.

Strategy:
- Data-parallel over tokens: 16384 tokens -> 8 shards of 2048.
- Router GEMM: host splits x and W into fp16 hi/lo pieces (x ~= xh + 2^-12*xm)
  and pre-transposes/permutes x so the device streams contiguous fat-descriptor
  DMAs. 3-term fp16 matmul (hH + 2^-12*(hM + mH)) accumulated in fp32 PSUM
  gives better-than-numpy-f32 logits at fp16 PE speed. The (wH|wM) columns are
  packed into PE column groups (tile_position) sharing one moving stream.
- logits^T [64, T] are PE-transposed back to [T(part), 64] tiles for
  softmax/argmax (free-dim reductions).
- One-hot = (logit == rowmax); capacity cumsum over the token dim via
  triangular-ones matmuls + a serial per-tile offset chain; cross-core
  segment offsets via an AllGather of per-core expert counts; aux loss from
  all-gathered count/prob sums.
- Per-group software pipeline: loads(g) | logit-transpose+softmax(g-1) |
  GEMM(g) | cumsum/counts matmuls(g-1), so the PE FIFO never waits on the
  DVE/ACT softmax chains.
"""

import os
import numpy as np
import ml_dtypes

import concourse.bacc as bacc
import concourse.bass as bass
import concourse.mybir as mybir
import concourse.tile as tile
from concourse import bass_utils

F32 = mybir.dt.float32
BF16 = mybir.dt.bfloat16
F16 = mybir.dt.float16
SC = float(2.0 ** -12)          # scale of the fp16 low pieces
FP16_MIN_NORMAL = 6.103515625e-05
AX = mybir.AxisListType
OP = mybir.AluOpType
ACTF = mybir.ActivationFunctionType

B, S, D, E = 4, 4096, 2048, 64
NCORES = 8
TOK = B * S                 # 16384
TPC = TOK // NCORES         # 2048 tokens per core
CAP = float(TOK) / E * 1.0  # 256.0
ALPHA = 0.01
NJ = D // 128               # 16 contraction chunks
NT = TPC // 128             # 16 token tiles per core
NG = 8                      # token groups per core
GT = TPC // NG              # 512 tokens per group
TPG = GT // 128             # 4 token tiles per group


def build_program(single_core=False):
    """single_core=True replaces the collective with a local DMA so the
    program can run under single-core simulators (timing analysis only)."""
    nc = bacc.Bacc("TRN2", target_bir_lowering=False, debug=False,
                   enable_asserts=True,
                   num_devices=1 if single_core else NCORES)

    # x pieces arrive host-permuted as [128(p), NG, NJ, GT]:
    # element [p, g, j, t] = x[g*GT + t, 128*j + p], so each partition row of a
    # per-group load is one contiguous 16 KB run in DRAM.
    xh = nc.dram_tensor("xh", [128, NG, NJ, GT], F16, kind="ExternalInput").ap()
    xm = nc.dram_tensor("xm", [128, NG, NJ, GT], F16, kind="ExternalInput").ap()
    # combined W pieces, host-permuted: [:, j, 0:64]=wH, [:, j, 64:128]=wM
    wHM = nc.dram_tensor("wHM", [128, NJ, 2 * E], F16, kind="ExternalInput").ap()
    bias = nc.dram_tensor("bias", [1, E], F32, kind="ExternalInput").ap()
    triu = nc.dram_tensor("triu", [128, 128], BF16, kind="ExternalInput").ap()
    ident64 = nc.dram_tensor("ident64", [64, 64], F32, kind="ExternalInput").ap()
    onesrow = nc.dram_tensor("onesrow", [1, 128], F32, kind="ExternalInput").ap()
    onescol = nc.dram_tensor("onescol", [128, 1], F32, kind="ExternalInput").ap()
    prevmask = nc.dram_tensor("prevmask", [NCORES, 1], F32, kind="ExternalInput").ap()

    out = nc.dram_tensor("out", [TPC, E], F32, kind="ExternalOutput").ap()
    aux = nc.dram_tensor("aux", [1, 1], F32, kind="ExternalOutput").ap()

    cc_in = nc.dram_tensor("cc_in", [1, 2 * E], F32, kind="Internal")
    cc_out = nc.dram_tensor("cc_out", [NCORES, 2 * E], F32, kind="Internal")

    with tile.TileContext(nc) as tc:
        with tc.tile_pool(name="consts", bufs=1) as consts, \
             tc.tile_pool(name="xt", bufs=3) as xtp, \
             tc.tile_pool(name="work", bufs=3) as work, \
             tc.tile_pool(name="soft", bufs=6) as soft, \
             tc.tile_pool(name="keep", bufs=NG) as keep, \
             tc.tile_pool(name="fin", bufs=1) as finp, \
             tc.tile_pool(name="plog", bufs=2, space="PSUM") as plog, \
             tc.tile_pool(name="psmall", bufs=2, space="PSUM") as psmall, \
             tc.tile_pool(name="pacc", bufs=2, space="PSUM") as pacc:

            # ---- group-0 x loads first (the DMA critical path) ----
            xth0 = xtp.tile([128, NJ, GT], F16, tag="xth", name="xth")
            nc.sync.dma_start(xth0[:], xh[:, 0, :, :])
            xtm0 = xtp.tile([128, NJ, GT], F16, tag="xtm", name="xtm")
            nc.sync.dma_start(xtm0[:], xm[:, 0, :, :])

            # ---- constants ----
            wHM_sb = consts.tile([128, NJ, 2 * E], F16)
            nc.sync.dma_start(wHM_sb[:], wHM)
            triu_sb = consts.tile([128, 128], BF16)
            nc.sync.dma_start(triu_sb[:], triu)
            id64_sb = consts.tile([64, 64], F32)
            nc.sync.dma_start(id64_sb[:], ident64)
            ones_r = consts.tile([1, 128], F32)
            nc.sync.dma_start(ones_r[:], onesrow)
            ones_c = consts.tile([128, 1], F32)
            nc.sync.dma_start(ones_c[:], onescol)
            pmask = consts.tile([NCORES, 1], F32)
            nc.sync.dma_start(pmask[:], prevmask)
            b1 = consts.tile([1, E], F32)
            nc.sync.dma_start(b1[:], bias)
            bB = consts.tile([128, E], F32)
            nc.gpsimd.partition_broadcast(bB[:], b1[:])

            # running per-expert counts: slot t holds counts before tile t
            offs = consts.tile([1, (NT + 1) * E], F32)
            nc.vector.memset(offs[0:1, 0:E], 0.0)

            # P_i accumulator (sum of probs over this core's tokens)
            p_P = pacc.tile([1, E], F32, tag="pacc")

            ru_k = {}    # per-group routed probs [128, TPG, E]
            cum_k = {}   # per-group local cumsum counts [128, TPG, E]
            state = {}   # per-tile softmax products needed by section B

            def section_a(g, ltB):
                """Logit re-transpose + full softmax/one-hot chain for each of
                group g's four 128-token tiles (PE work is just 4 transposes;
                the rest streams on DVE/ACT while the next GEMM runs)."""
                for i in range(TPG):
                    t = g * TPG + i
                    sl = slice(i * 128, (i + 1) * 128)

                    p_lg = psmall.tile([128, E], F32, tag="psmall", name="p_lg")
                    nc.tensor.transpose(p_lg[:], ltB[:, sl], id64_sb[:])

                    lg = soft.tile([128, E], F32, tag="lg", name="lg")
                    nc.vector.tensor_tensor(lg[:], p_lg[:], bB[:], op=OP.add)

                    m = soft.tile([128, 1], F32, tag="m", name="m")
                    nc.vector.reduce_max(m[:], lg[:], axis=AX.X, negate=True)
                    ex = soft.tile([128, E], F32, tag="ex", name="ex")
                    ssum = soft.tile([128, 1], F32, tag="ssum", name="ssum")
                    nc.scalar.activation(ex[:], lg[:], ACTF.Exp,
                                         bias=m[:], scale=1.0, accum_out=ssum[:])
                    rcp = soft.tile([128, 1], F32, tag="rcp", name="rcp")
                    nc.vector.reciprocal(rcp[:], ssum[:])
                    probs = soft.tile([128, E], F32, tag="probs", name="probs")
                    nc.vector.tensor_scalar(probs[:], ex[:], rcp[:], None,
                                            op0=OP.mult)

                    # one-hot of argmax: (logit + (-max)) == 0
                    oh = soft.tile([128, E], BF16, tag="oh", name="oh")
                    nc.vector.tensor_scalar(oh[:], lg[:], m[:], 0.0,
                                            op0=OP.add, op1=OP.is_equal)

                    # routed prob = probs * onehot (kept for phase 3)
                    if i == 0:
                        ru_k[g] = keep.tile([128, TPG, E], F32, tag="ru",
                                            name="ru", bufs=NG)
                    nc.vector.tensor_tensor(ru_k[g][:, i, :], probs[:],
                                            oh[:], op=OP.mult)
                    state[t] = (probs, oh)

            def section_b(g):
                """Count/cumsum matmuls for group g (all DVE/ACT inputs are
                ready by the time the PE FIFO reaches these)."""
                for i in range(TPG):
                    t = g * TPG + i
                    probs, oh = state.pop(t)

                    # P_i partial sums: ones^T @ probs accumulated over tiles
                    nc.tensor.matmul(p_P[:], ones_c[:], probs[:],
                                     start=(t == 0), stop=(t == NT - 1))

                    # local cumsum: triu^T (prefix) + broadcast of offs[t]
                    p_cum = psmall.tile([128, E], F32, tag="psmall",
                                        name="p_cum")
                    nc.tensor.matmul(p_cum[:], triu_sb[:], oh[:],
                                     start=True, stop=False)
                    nc.tensor.matmul(p_cum[:], ones_r[:],
                                     offs[0:1, t * E:(t + 1) * E],
                                     start=False, stop=True)
                    # next tile's offset = offs[t] + this tile's counts
                    # (ones^T @ oh; triu's last column is all-ones)
                    p_cs = pacc.tile([1, E], F32, tag="pacc", name="p_cs")
                    nc.tensor.matmul(p_cs[:], triu_sb[:, 127:128], oh[:],
                                     start=True, stop=True)
                    nc.vector.tensor_tensor(offs[0:1, (t + 1) * E:(t + 2) * E],
                                            offs[0:1, t * E:(t + 1) * E],
                                            p_cs[:], op=OP.add)
                    if i == 0:
                        cum_k[g] = keep.tile([128, TPG, E], F32, tag="cum",
                                             name="cum", bufs=NG)
                    nc.scalar.copy(cum_k[g][:, i, :], p_cum[:])

            prev = None
            for g in range(NG):
                # ---- fat-descriptor loads, split in j-halves ----
                if g == 0:
                    xth, xtm = xth0, xtm0
                else:
                    xth = xtp.tile([128, NJ, GT], F16, tag="xth", name="xth")
                    nc.sync.dma_start(xth[:], xh[:, g, :, :])
                    xtm = xtp.tile([128, NJ, GT], F16, tag="xtm", name="xtm")
                    nc.sync.dma_start(xtm[:], xm[:, g, :, :])

                if prev is not None:
                    section_a(prev[0], prev[1])

                # ---- 3-term GEMM: logits = hH + SC*(hM + mH) ----
                # pA = [wH|wM].T @ xh  (rows 0:64 = hH, rows 64:128 = hM)
                # pB = [wH|wM].T @ xm  (rows 0:64 = mH, rows 64:128 unused mM)
                pA = plog.tile([128, GT], F32, tag="pA")
                pB = plog.tile([128, GT], F32, tag="pB")
                for j in range(NJ):
                    nc.tensor.matmul(pA[:, :], wHM_sb[:, j, :], xth[:, j, :],
                                     start=(j == 0), stop=(j == NJ - 1))
                ltA = work.tile([64, GT], F32, tag="ltA")
                nc.scalar.copy(ltA[:], pA[64:128, :])
                for j in range(NJ):
                    nc.tensor.matmul(pB[:, :], wHM_sb[:, j, :],
                                     xtm[:, j, :], start=(j == 0),
                                     stop=(j == NJ - 1))
                # combine: ltB = (ltA + pB_mH)*SC + pA_hH
                ltS = work.tile([64, GT], F32, tag="ltS")
                nc.vector.tensor_tensor(ltS[:], ltA[:], pB[0:64, :], op=OP.add)
                ltB = work.tile([64, GT], F32, tag="ltB")
                nc.vector.scalar_tensor_tensor(ltB[:], ltS[:], SC, pA[0:64, :],
                                               op0=OP.mult, op1=OP.add)
                if prev is not None:
                    section_b(prev[0])
                prev = (g, ltB)
            section_a(prev[0], prev[1])
            section_b(prev[0])

            # ---- cross-core exchange: [counts | probsums] ----
            stats = work.tile([1, 2 * E], F32, tag="stats")
            nc.vector.tensor_copy(stats[0:1, 0:E], offs[0:1, NT * E:(NT + 1) * E])
            nc.vector.tensor_copy(stats[0:1, E:2 * E], p_P[:])
            nc.sync.dma_start(cc_in.ap(), stats[:])
            if single_core:
                nc.sync.dma_start(cc_out.ap()[0:1, :], cc_in.ap())
            else:
                nc.gpsimd.collective_compute(
                    "AllGather", OP.bypass,
                    replica_groups=[list(range(NCORES))],
                    ins=[cc_in.ap()], outs=[cc_out.ap()])
            gath = work.tile([NCORES, 2 * E], F32, tag="gath")
            nc.sync.dma_start(gath[:], cc_out.ap())

            # per-core segment offset = sum of previous cores' counts
            p_off = pacc.tile([1, 2 * E], F32, tag="pacc")
            nc.tensor.matmul(p_off[:], pmask[:], gath[:], start=True, stop=True)
            coreoff = work.tile([1, TPG * E], F32, tag="coreoff")
            for i in range(TPG):
                nc.vector.tensor_copy(coreoff[0:1, i * E:(i + 1) * E],
                                      p_off[0:1, 0:E])
            offB = work.tile([128, TPG * E], F32, tag="offB")
            nc.gpsimd.partition_broadcast(offB[:], coreoff[:])
            # capB = CAP - coreoff (per-expert threshold, TPG-wide)
            capB = work.tile([128, TPG, E], F32, tag="capB")
            nc.vector.tensor_scalar(
                capB.bitcast(F32).rearrange("p (i e) -> p i e", e=E)
                if False else capB[:],
                offB[:].rearrange("p (i e) -> p i e", e=E), -1.0, CAP,
                op0=OP.mult, op1=OP.add)

            # aux loss = ALPHA*E * sum(counts/TOK * probsum/TOK)
            p_tot = pacc.tile([1, 2 * E], F32, tag="pacc")
            nc.tensor.matmul(p_tot[:], ones_c[0:NCORES, :], gath[:],
                             start=True, stop=True)
            tots = work.tile([1, 2 * E], F32, tag="tots")
            nc.vector.tensor_copy(tots[:], p_tot[:])
            fp = work.tile([1, E], F32, tag="fp")
            nc.vector.tensor_tensor(fp[:], tots[0:1, 0:E], tots[0:1, E:2 * E],
                                    op=OP.mult)
            auxv = work.tile([1, 1], F32, tag="auxv")
            nc.vector.reduce_sum(auxv[:], fp[:], axis=AX.X)
            aux_sb = work.tile([1, 1], F32, tag="aux_sb")
            nc.vector.tensor_scalar(aux_sb[:], auxv[:],
                                    float(ALPHA * E / (TOK * TOK)), None,
                                    op0=OP.mult)
            nc.sync.dma_start(aux, aux_sb[:])

            # ---- capacity mask + final output (one op pair per group) ----
            fin = finp.tile([128, NT, E], F32)
            out_r = out.rearrange("(t p) e -> p t e", p=128)
            for g in range(NG):
                # mask = (cum <= CAP - coreoff), applied to routed probs
                msk = work.tile([128, TPG, E], F32, tag="msk")
                nc.vector.scalar_tensor_tensor(msk[:], cum_k[g][:], 0.0,
                                               capB[:], op0=OP.add,
                                               op1=OP.is_le)
                nc.vector.tensor_tensor(fin[:, g * TPG:(g + 1) * TPG, :],
                                        msk[:], ru_k[g][:], op=OP.mult)
                nc.sync.dma_start(out_r[:, g * TPG:(g + 1) * TPG, :],
                                  fin[:, g * TPG:(g + 1) * TPG, :])

    nc.compile()
    return nc


_CACHE = {}


def _get_program():
    if "nc" not in _CACHE:
        _CACHE["nc"] = build_program()
    return _CACHE["nc"]


def _split_fp16(a):
    """a ~= ah + 2^-12 * am, both fp16, subnormals flushed host-side."""
    ah = a.astype(np.float16).astype(np.float32)
    ah[np.abs(ah) < FP16_MIN_NORMAL] = 0.0
    ah16 = ah.astype(np.float16)
    am = ((a - ah) * float(2.0 ** 12)).astype(np.float16).astype(np.float32)
    am[np.abs(am) < FP16_MIN_NORMAL] = 0.0
    return ah16, am.astype(np.float16)


def _dev_layout(piece):
    """[TPC, D] fp16 shard piece -> [128, NG, NJ, GT] device layout."""
    return np.ascontiguousarray(
        piece.reshape(NG, GT, NJ, 128).transpose(3, 0, 2, 1))


def _prep_inputs(x, W, b):
    bf = ml_dtypes.bfloat16
    xf = np.ascontiguousarray(np.asarray(x).reshape(TOK, D)).astype(np.float32)
    xh, xm = _split_fp16(xf)
    Wf = np.asarray(W, dtype=np.float32)
    wH, wM = _split_fp16(Wf)
    wH = wH.reshape(NJ, 128, E).transpose(1, 0, 2)
    wM = wM.reshape(NJ, 128, E).transpose(1, 0, 2)
    wHM = np.ascontiguousarray(np.concatenate([wH, wM], axis=2))
    bias = np.asarray(b, dtype=np.float32).reshape(1, E)
    triu = np.triu(np.ones((128, 128), dtype=np.float32)).astype(bf)
    ident64 = np.eye(64, dtype=np.float32)
    onesrow = np.ones((1, 128), dtype=np.float32)
    onescol = np.ones((128, 1), dtype=np.float32)

    in_maps = []
    for c in range(NCORES):
        pm = np.zeros((NCORES, 1), dtype=np.float32)
        pm[:c] = 1.0
        sl = slice(c * TPC, (c + 1) * TPC)
        in_maps.append({
            "xh": _dev_layout(xh[sl]),
            "xm": _dev_layout(xm[sl]),
            "wHM": wHM, "bias": bias, "triu": triu,
            "ident64": ident64, "onesrow": onesrow, "onescol": onescol,
            "prevmask": pm,
        })
    return in_maps


def run(x, W, b, trace=False, trace_cores=None):
    nc = _get_program()
    in_maps = _prep_inputs(x, W, b)
    kw = {}
    if trace_cores is not None:
        kw["trace_cores"] = trace_cores
    res = bass_utils.run_bass_kernel_spmd(
        nc, in_maps, core_ids=list(range(NCORES)), trace=trace, **kw)
    outs = np.concatenate([res.results[c]["out"] for c in range(NCORES)], axis=0)
    routed = outs.reshape(B, S, E).astype(np.float32)
    aux_loss = np.float32(res.results[0]["aux"][0, 0])
    return (routed, aux_loss), res


def kernel(x, W, b):
    (routed, aux_loss), _ = run(x, W, b, trace=False)
    return routed, aux_loss


# revision 15
# speedup vs baseline: 1.1032x; 1.1032x over previous
"""MoE top-1 routing with expert capacity (nn_ExpertAllocation) on 8 TRN2 cores.

Strategy:
- Data-parallel over tokens: 16384 tokens -> 8 shards of 2048.
- Router GEMM: host splits x and W into fp16 hi/lo pieces (x ~= xh + 2^-12*xm)
  and pre-transposes/permutes x so the device streams contiguous fat-descriptor
  DMAs. 3-term fp16 matmul (hH + 2^-12*(hM + mH)) accumulated in fp32 PSUM
  gives better-than-numpy-f32 logits at fp16 PE speed. The (wH|wM) columns are
  packed into PE column groups (tile_position) sharing one moving stream.
- logits^T [64, T] are PE-transposed back to [T(part), 64] tiles for
  softmax/argmax (free-dim reductions).
- One-hot = (logit == rowmax); capacity cumsum over the token dim via
  triangular-ones matmuls + a serial per-tile offset chain; cross-core
  segment offsets via an AllGather of per-core expert counts; aux loss from
  all-gathered count/prob sums.
- Per-group software pipeline: loads(g) | logit-transpose+softmax(g-1) |
  GEMM(g) | cumsum/counts matmuls(g-1), so the PE FIFO never waits on the
  DVE/ACT softmax chains.
"""

import os
import numpy as np
import ml_dtypes

import concourse.bacc as bacc
import concourse.bass as bass
import concourse.mybir as mybir
import concourse.tile as tile
from concourse import bass_utils

F32 = mybir.dt.float32
BF16 = mybir.dt.bfloat16
F16 = mybir.dt.float16
SC = float(2.0 ** -12)          # scale of the fp16 low pieces
FP16_MIN_NORMAL = 6.103515625e-05
AX = mybir.AxisListType
OP = mybir.AluOpType
ACTF = mybir.ActivationFunctionType

B, S, D, E = 4, 4096, 2048, 64
NCORES = 8
TOK = B * S                 # 16384
TPC = TOK // NCORES         # 2048 tokens per core
CAP = float(TOK) / E * 1.0  # 256.0
ALPHA = 0.01
NJ = D // 128               # 16 contraction chunks
NT = TPC // 128             # 16 token tiles per core
NG = 8                      # token groups per core
GT = TPC // NG              # 512 tokens per group
TPG = GT // 128             # 4 token tiles per group


def build_program(single_core=False):
    """single_core=True replaces the collective with a local DMA so the
    program can run under single-core simulators (timing analysis only)."""
    nc = bacc.Bacc("TRN2", target_bir_lowering=False, debug=False,
                   enable_asserts=True,
                   num_devices=1 if single_core else NCORES)

    # x pieces arrive host-permuted as [128(p), NG, NJ, GT]:
    # element [p, g, j, t] = x[g*GT + t, 128*j + p], so each partition row of a
    # per-group load is one contiguous 16 KB run in DRAM.
    xh = nc.dram_tensor("xh", [128, NG, NJ, GT], F16, kind="ExternalInput").ap()
    xm = nc.dram_tensor("xm", [128, NG, NJ, GT], F16, kind="ExternalInput").ap()
    # combined W pieces, host-permuted: [:, j, 0:64]=wH, [:, j, 64:128]=wM
    wHM = nc.dram_tensor("wHM", [128, NJ, 2 * E], F16, kind="ExternalInput").ap()
    bias = nc.dram_tensor("bias", [1, E], F32, kind="ExternalInput").ap()
    triu = nc.dram_tensor("triu", [128, 128], BF16, kind="ExternalInput").ap()
    ident64 = nc.dram_tensor("ident64", [64, 64], F32, kind="ExternalInput").ap()
    onesrow = nc.dram_tensor("onesrow", [1, 128], F32, kind="ExternalInput").ap()
    onescol = nc.dram_tensor("onescol", [128, 1], F32, kind="ExternalInput").ap()
    prevmask = nc.dram_tensor("prevmask", [NCORES, 1], F32, kind="ExternalInput").ap()

    out = nc.dram_tensor("out", [TPC, E], F32, kind="ExternalOutput").ap()
    aux = nc.dram_tensor("aux", [1, 1], F32, kind="ExternalOutput").ap()

    cc_in = nc.dram_tensor("cc_in", [1, 2 * E], F32, kind="Internal")
    cc_out = nc.dram_tensor("cc_out", [NCORES, 2 * E], F32, kind="Internal")

    with tile.TileContext(nc) as tc:
        with tc.tile_pool(name="consts", bufs=1) as consts, \
             tc.tile_pool(name="xt", bufs=3) as xtp, \
             tc.tile_pool(name="work", bufs=3) as work, \
             tc.tile_pool(name="soft", bufs=6) as soft, \
             tc.tile_pool(name="keep", bufs=NG) as keep, \
             tc.tile_pool(name="fin", bufs=1) as finp, \
             tc.tile_pool(name="plog", bufs=2, space="PSUM") as plog, \
             tc.tile_pool(name="psmall", bufs=2, space="PSUM") as psmall, \
             tc.tile_pool(name="pacc", bufs=2, space="PSUM") as pacc:

            # ---- group-0 x loads first (the DMA critical path) ----
            xth0 = xtp.tile([128, NJ, GT], F16, tag="xth", name="xth")
            nc.sync.dma_start(xth0[:, 0:NJ // 2, :], xh[:, 0, 0:NJ // 2, :])
            nc.sync.dma_start(xth0[:, NJ // 2:, :], xh[:, 0, NJ // 2:, :])
            xtm0 = xtp.tile([128, NJ, GT], F16, tag="xtm", name="xtm")
            nc.sync.dma_start(xtm0[:, 0:NJ // 2, :], xm[:, 0, 0:NJ // 2, :])
            nc.sync.dma_start(xtm0[:, NJ // 2:, :], xm[:, 0, NJ // 2:, :])

            # ---- constants ----
            wHM_sb = consts.tile([128, NJ, 2 * E], F16)
            nc.sync.dma_start(wHM_sb[:], wHM)
            triu_sb = consts.tile([128, 128], BF16)
            nc.sync.dma_start(triu_sb[:], triu)
            id64_sb = consts.tile([64, 64], F32)
            nc.sync.dma_start(id64_sb[:], ident64)
            ones_r = consts.tile([1, 128], F32)
            nc.sync.dma_start(ones_r[:], onesrow)
            ones_c = consts.tile([128, 1], F32)
            nc.sync.dma_start(ones_c[:], onescol)
            pmask = consts.tile([NCORES, 1], F32)
            nc.sync.dma_start(pmask[:], prevmask)
            b1 = consts.tile([1, E], F32)
            nc.sync.dma_start(b1[:], bias)
            bB = consts.tile([128, E], F32)
            nc.gpsimd.partition_broadcast(bB[:], b1[:])

            # running per-expert counts: slot t holds counts before tile t
            offs = consts.tile([1, (NT + 1) * E], F32)
            nc.vector.memset(offs[0:1, 0:E], 0.0)

            # P_i accumulator (sum of probs over this core's tokens)
            p_P = pacc.tile([1, E], F32, tag="pacc")

            ru_k = {}    # per-group routed probs [128, TPG, E]
            cum_k = {}   # per-group local cumsum counts [128, TPG, E]
            state = {}   # per-tile softmax products needed by section B

            def section_a(g, ltB):
                """Logit re-transpose + full softmax/one-hot chain for each of
                group g's four 128-token tiles (PE work is just 4 transposes;
                the rest streams on DVE/ACT while the next GEMM runs)."""
                for i in range(TPG):
                    t = g * TPG + i
                    sl = slice(i * 128, (i + 1) * 128)

                    p_lg = psmall.tile([128, E], F32, tag="psmall", name="p_lg")
                    nc.tensor.transpose(p_lg[:], ltB[:, sl], id64_sb[:])

                    lg = soft.tile([128, E], F32, tag="lg", name="lg")
                    nc.vector.tensor_tensor(lg[:], p_lg[:], bB[:], op=OP.add)

                    m = soft.tile([128, 1], F32, tag="m", name="m")
                    nc.vector.reduce_max(m[:], lg[:], axis=AX.X, negate=True)
                    ex = soft.tile([128, E], F32, tag="ex", name="ex")
                    ssum = soft.tile([128, 1], F32, tag="ssum", name="ssum")
                    nc.scalar.activation(ex[:], lg[:], ACTF.Exp,
                                         bias=m[:], scale=1.0, accum_out=ssum[:])
                    rcp = soft.tile([128, 1], F32, tag="rcp", name="rcp")
                    nc.vector.reciprocal(rcp[:], ssum[:])
                    probs = soft.tile([128, E], F32, tag="probs", name="probs")
                    nc.vector.tensor_scalar(probs[:], ex[:], rcp[:], None,
                                            op0=OP.mult)

                    # one-hot of argmax: (logit + (-max)) == 0
                    oh = soft.tile([128, E], BF16, tag="oh", name="oh")
                    nc.vector.tensor_scalar(oh[:], lg[:], m[:], 0.0,
                                            op0=OP.add, op1=OP.is_equal)

                    # routed prob = probs * onehot (kept for phase 3)
                    if i == 0:
                        ru_k[g] = keep.tile([128, TPG, E], F32, tag="ru",
                                            name="ru", bufs=NG)
                    nc.vector.tensor_tensor(ru_k[g][:, i, :], probs[:],
                                            oh[:], op=OP.mult)
                    state[t] = (probs, oh)

            def section_b(g):
                """Count/cumsum matmuls for group g (all DVE/ACT inputs are
                ready by the time the PE FIFO reaches these)."""
                for i in range(TPG):
                    t = g * TPG + i
                    probs, oh = state.pop(t)

                    # P_i partial sums: ones^T @ probs accumulated over tiles
                    nc.tensor.matmul(p_P[:], ones_c[:], probs[:],
                                     start=(t == 0), stop=(t == NT - 1))

                    # local cumsum: triu^T (prefix) + broadcast of offs[t]
                    p_cum = psmall.tile([128, E], F32, tag="psmall",
                                        name="p_cum")
                    nc.tensor.matmul(p_cum[:], triu_sb[:], oh[:],
                                     start=True, stop=False)
                    nc.tensor.matmul(p_cum[:], ones_r[:],
                                     offs[0:1, t * E:(t + 1) * E],
                                     start=False, stop=True)
                    # next tile's offset = offs[t] + this tile's counts
                    # (ones^T @ oh; triu's last column is all-ones)
                    p_cs = pacc.tile([1, E], F32, tag="pacc", name="p_cs")
                    nc.tensor.matmul(p_cs[:], triu_sb[:, 127:128], oh[:],
                                     start=True, stop=True)
                    nc.vector.tensor_tensor(offs[0:1, (t + 1) * E:(t + 2) * E],
                                            offs[0:1, t * E:(t + 1) * E],
                                            p_cs[:], op=OP.add)
                    if i == 0:
                        cum_k[g] = keep.tile([128, TPG, E], F32, tag="cum",
                                             name="cum", bufs=NG)
                    nc.scalar.copy(cum_k[g][:, i, :], p_cum[:])

            prev = None
            for g in range(NG):
                # ---- fat-descriptor loads, split in j-halves ----
                if g == 0:
                    xth, xtm = xth0, xtm0
                else:
                    xth = xtp.tile([128, NJ, GT], F16, tag="xth", name="xth")
                    nc.sync.dma_start(xth[:, 0:NJ // 2, :],
                                      xh[:, g, 0:NJ // 2, :])
                    nc.sync.dma_start(xth[:, NJ // 2:, :],
                                      xh[:, g, NJ // 2:, :])
                    xtm = xtp.tile([128, NJ, GT], F16, tag="xtm", name="xtm")
                    nc.sync.dma_start(xtm[:, 0:NJ // 2, :],
                                      xm[:, g, 0:NJ // 2, :])
                    nc.sync.dma_start(xtm[:, NJ // 2:, :],
                                      xm[:, g, NJ // 2:, :])

                if prev is not None:
                    section_a(prev[0], prev[1])

                # ---- 3-term GEMM: logits = hH + SC*(hM + mH) ----
                # pA = [wH|wM].T @ xh  (rows 0:64 = hH, rows 64:128 = hM)
                # pB = [wH|wM].T @ xm  (rows 0:64 = mH, rows 64:128 unused mM)
                pA = plog.tile([128, GT], F32, tag="pA")
                pB = plog.tile([128, GT], F32, tag="pB")
                for j in range(NJ):
                    nc.tensor.matmul(pA[:, :], wHM_sb[:, j, :], xth[:, j, :],
                                     start=(j == 0), stop=(j == NJ - 1))
                ltA = work.tile([64, GT], F32, tag="ltA")
                nc.scalar.copy(ltA[:], pA[64:128, :])
                for j in range(NJ):
                    nc.tensor.matmul(pB[:, :], wHM_sb[:, j, :],
                                     xtm[:, j, :], start=(j == 0),
                                     stop=(j == NJ - 1))
                # combine: ltB = (ltA + pB_mH)*SC + pA_hH
                ltS = work.tile([64, GT], F32, tag="ltS")
                nc.vector.tensor_tensor(ltS[:], ltA[:], pB[0:64, :], op=OP.add)
                ltB = work.tile([64, GT], F32, tag="ltB")
                nc.vector.scalar_tensor_tensor(ltB[:], ltS[:], SC, pA[0:64, :],
                                               op0=OP.mult, op1=OP.add)
                if prev is not None:
                    section_b(prev[0])
                prev = (g, ltB)
            section_a(prev[0], prev[1])
            section_b(prev[0])

            # ---- cross-core exchange: [counts | probsums] ----
            stats = work.tile([1, 2 * E], F32, tag="stats")
            nc.vector.tensor_copy(stats[0:1, 0:E], offs[0:1, NT * E:(NT + 1) * E])
            nc.vector.tensor_copy(stats[0:1, E:2 * E], p_P[:])
            nc.sync.dma_start(cc_in.ap(), stats[:])
            if single_core:
                nc.sync.dma_start(cc_out.ap()[0:1, :], cc_in.ap())
            else:
                nc.gpsimd.collective_compute(
                    "AllGather", OP.bypass,
                    replica_groups=[list(range(NCORES))],
                    ins=[cc_in.ap()], outs=[cc_out.ap()])
            gath = work.tile([NCORES, 2 * E], F32, tag="gath")
            nc.sync.dma_start(gath[:], cc_out.ap())

            # per-core segment offset = sum of previous cores' counts
            p_off = pacc.tile([1, 2 * E], F32, tag="pacc")
            nc.tensor.matmul(p_off[:], pmask[:], gath[:], start=True, stop=True)
            coreoff = work.tile([1, TPG * E], F32, tag="coreoff")
            for i in range(TPG):
                nc.vector.tensor_copy(coreoff[0:1, i * E:(i + 1) * E],
                                      p_off[0:1, 0:E])
            offB = work.tile([128, TPG * E], F32, tag="offB")
            nc.gpsimd.partition_broadcast(offB[:], coreoff[:])
            # capB = CAP - coreoff (per-expert threshold, TPG-wide)
            capB = work.tile([128, TPG, E], F32, tag="capB")
            nc.vector.tensor_scalar(
                capB.bitcast(F32).rearrange("p (i e) -> p i e", e=E)
                if False else capB[:],
                offB[:].rearrange("p (i e) -> p i e", e=E), -1.0, CAP,
                op0=OP.mult, op1=OP.add)

            # aux loss = ALPHA*E * sum(counts/TOK * probsum/TOK)
            p_tot = pacc.tile([1, 2 * E], F32, tag="pacc")
            nc.tensor.matmul(p_tot[:], ones_c[0:NCORES, :], gath[:],
                             start=True, stop=True)
            tots = work.tile([1, 2 * E], F32, tag="tots")
            nc.vector.tensor_copy(tots[:], p_tot[:])
            fp = work.tile([1, E], F32, tag="fp")
            nc.vector.tensor_tensor(fp[:], tots[0:1, 0:E], tots[0:1, E:2 * E],
                                    op=OP.mult)
            auxv = work.tile([1, 1], F32, tag="auxv")
            nc.vector.reduce_sum(auxv[:], fp[:], axis=AX.X)
            aux_sb = work.tile([1, 1], F32, tag="aux_sb")
            nc.vector.tensor_scalar(aux_sb[:], auxv[:],
                                    float(ALPHA * E / (TOK * TOK)), None,
                                    op0=OP.mult)
            nc.sync.dma_start(aux, aux_sb[:])

            # ---- capacity mask + final output (one op pair per group) ----
            fin = finp.tile([128, NT, E], F32)
            out_r = out.rearrange("(t p) e -> p t e", p=128)
            for g in range(NG):
                # mask = (cum <= CAP - coreoff), applied to routed probs
                msk = work.tile([128, TPG, E], F32, tag="msk")
                nc.vector.scalar_tensor_tensor(msk[:], cum_k[g][:], 0.0,
                                               capB[:], op0=OP.add,
                                               op1=OP.is_le)
                nc.vector.tensor_tensor(fin[:, g * TPG:(g + 1) * TPG, :],
                                        msk[:], ru_k[g][:], op=OP.mult)
                nc.sync.dma_start(out_r[:, g * TPG:(g + 1) * TPG, :],
                                  fin[:, g * TPG:(g + 1) * TPG, :])

    nc.compile()
    return nc


_CACHE = {}


def _get_program():
    if "nc" not in _CACHE:
        _CACHE["nc"] = build_program()
    return _CACHE["nc"]


def _split_fp16(a):
    """a ~= ah + 2^-12 * am, both fp16, subnormals flushed host-side."""
    ah = a.astype(np.float16).astype(np.float32)
    ah[np.abs(ah) < FP16_MIN_NORMAL] = 0.0
    ah16 = ah.astype(np.float16)
    am = ((a - ah) * float(2.0 ** 12)).astype(np.float16).astype(np.float32)
    am[np.abs(am) < FP16_MIN_NORMAL] = 0.0
    return ah16, am.astype(np.float16)


def _dev_layout(piece):
    """[TPC, D] fp16 shard piece -> [128, NG, NJ, GT] device layout."""
    return np.ascontiguousarray(
        piece.reshape(NG, GT, NJ, 128).transpose(3, 0, 2, 1))


def _prep_inputs(x, W, b):
    bf = ml_dtypes.bfloat16
    xf = np.ascontiguousarray(np.asarray(x).reshape(TOK, D)).astype(np.float32)
    xh, xm = _split_fp16(xf)
    Wf = np.asarray(W, dtype=np.float32)
    wH, wM = _split_fp16(Wf)
    wH = wH.reshape(NJ, 128, E).transpose(1, 0, 2)
    wM = wM.reshape(NJ, 128, E).transpose(1, 0, 2)
    wHM = np.ascontiguousarray(np.concatenate([wH, wM], axis=2))
    bias = np.asarray(b, dtype=np.float32).reshape(1, E)
    triu = np.triu(np.ones((128, 128), dtype=np.float32)).astype(bf)
    ident64 = np.eye(64, dtype=np.float32)
    onesrow = np.ones((1, 128), dtype=np.float32)
    onescol = np.ones((128, 1), dtype=np.float32)

    in_maps = []
    for c in range(NCORES):
        pm = np.zeros((NCORES, 1), dtype=np.float32)
        pm[:c] = 1.0
        sl = slice(c * TPC, (c + 1) * TPC)
        in_maps.append({
            "xh": _dev_layout(xh[sl]),
            "xm": _dev_layout(xm[sl]),
            "wHM": wHM, "bias": bias, "triu": triu,
            "ident64": ident64, "onesrow": onesrow, "onescol": onescol,
            "prevmask": pm,
        })
    return in_maps


def run(x, W, b, trace=False, trace_cores=None):
    nc = _get_program()
    in_maps = _prep_inputs(x, W, b)
    kw = {}
    if trace_cores is not None:
        kw["trace_cores"] = trace_cores
    res = bass_utils.run_bass_kernel_spmd(
        nc, in_maps, core_ids=list(range(NCORES)), trace=trace, **kw)
    outs = np.concatenate([res.results[c]["out"] for c in range(NCORES)], axis=0)
    routed = outs.reshape(B, S, E).astype(np.float32)
    aux_loss = np.float32(res.results[0]["aux"][0, 0])
    return (routed, aux_loss), res


def kernel(x, W, b):
    (routed, aux_loss), _ = run(x, W, b, trace=False)
    return routed, aux_loss
